# revision 1
# baseline (speedup 1.0000x reference)
"""ArcFace loss on 8 TRN2 NeuronCores (batch-parallel Bass/Tile kernel).

Math: for non-target classes cos(arccos(x)) == x, so logits are just
SCALE*x everywhere except the B target entries, which get
SCALE*(x*cos(m) - sqrt(1-x^2)*sin(m)).  Since cosine < 0.99 strictly,
SCALE*0.99 is an upper bound of every logit, so a constant shift
K = SCALE*0.99 replaces the per-row max (logsumexp is shift-invariant)
and the [B, C] pass is a single streamed exp-accumulate:

    S_all[b]  = sum_c exp(SCALE*x[b,c] - K)           (device, streamed)
    lt[b]     = SCALE*(xt*cos(m) - sqrt(1-xt^2)*sin(m))
    S_true[b] = S_all - exp(SCALE*xt - K) + exp(lt - K)
    loss      = mean_b [ log(S_true) + K - lt ]

Sharding: batch dimension B=2048 -> 256 rows per core (zero-copy host
shards).  Each core streams its [256, 100000] f32 shard (102.4 MB)
through SBUF; ScalarE does exp + free-axis accumulation in a single
ACTIVATE per tile (accum_out), so the pass is purely DMA-bound
(~358 GB/s/core HBM -> ~286 us floor).  The margin correction for the
core's rows is computed up front (overlapped with streaming, and
keeping the Sqrt/Exp ACT-table switches out of the tail), the local
rows reduce to a partial mean, and a 4-byte AllReduce(add) combines
the 8 partial means.
"""

import math

import numpy as np

B = 2048
C = 100000
N_CORES = 8
B_PER = B // N_CORES  # 256 rows per core
RB = B_PER // 128  # 2 row-blocks of 128 partitions
CT = 16  # col-tiles per row-block
F = C // CT  # free dim per tile

MARGIN = 0.1
SCALE = 64.0
K_SHIFT = SCALE * 0.99  # upper bound of all logits; constant lse shift

_CACHE = {}


def build_bass(
    b_per=B_PER,
    c=C,
    ct=CT,
    n_cores=N_CORES,
    bufs=4,
    split_dma_engines=False,
    warmup_collective=True,
    final_collective=True,
    allgather_final=False,
):
    """Build + compile the SPMD Bass graph for one core (all cores identical)."""
    import concourse.bacc as bacc
    import concourse.bass as bass
    import concourse.tile as tile
    from concourse import mybir

    f32 = mybir.dt.float32
    AF = mybir.ActivationFunctionType
    rb = b_per // 128
    f = c // ct
    cos_m = float(np.float32(math.cos(MARGIN)))
    sin_m = float(np.float32(math.sin(MARGIN)))

    nc = bacc.Bacc(
        "TRN2",
        target_bir_lowering=False,
        debug=False,
        num_devices=n_cores,
    )
    cos_ext = nc.dram_tensor("cosine", [b_per, c], f32, kind="ExternalInput")
    xt_ext = nc.dram_tensor("xt", [128, rb], f32, kind="ExternalInput")
    out_ext = nc.dram_tensor("out", [1, 1], f32, kind="ExternalOutput")

    with tile.TileContext(nc) as tc:
        with (
            tc.tile_pool(name="stream", bufs=bufs) as stream_pool,
            tc.tile_pool(name="small", bufs=1) as small,
            tc.tile_pool(name="psum", bufs=1, space="PSUM") as psum,
            tc.tile_pool(name="dram", bufs=1, space="DRAM") as dram,
        ):
            # per-(row-block, col-tile) partial row sums from ACT accum_out;
            # one extra column per row-block holds the margin correction so
            # a single reduce yields S_true directly.
            acc = small.tile([128, rb * (ct + 1)], f32)

            # constant bias AP for exp(x*scale - K)
            kbias = small.tile([128, 1], f32)
            nc.vector.memset(kbias[:], -K_SHIFT)
            # matmul ones vector carries the 1/B mean scaling
            ones = small.tile([128, 1], f32)
            nc.vector.memset(ones[:], 1.0 / float(n_cores * b_per))

            if warmup_collective and final_collective:
                # fire a dummy AllReduce at t~0 so the ncfw collective
                # firmware is warm when the real one triggers in the tail;
                # completes during the stream, nothing waits on it.
                warm_sb = small.tile([1, 1], f32)
                nc.vector.memset(warm_sb[:], 0.0)
                warm_in = dram.tile([1, 1], f32)
                warm_out = dram.tile([1, 1], f32)
                nc.sync.dma_start(out=warm_in[:], in_=warm_sb[:])
                nc.gpsimd.collective_compute(
                    "AllReduce",
                    mybir.AluOpType.add,
                    replica_groups=[list(range(n_cores))],
                    ins=[warm_in.opt()],
                    outs=[warm_out.opt()],
                )

            # ---- epilogue head: margin terms (independent of the stream);
            # runs first so Sqrt's and Exp's ACT table loads stay out of
            # the tail and the work overlaps the first stream DMA.
            xt_sb = small.tile([128, rb], f32)
            nc.gpsimd.dma_start(out=xt_sb[:], in_=xt_ext[:])
            sq = small.tile([128, rb], f32)
            nc.vector.tensor_mul(sq[:], xt_sb[:], xt_sb[:])
            rt = small.tile([128, rb], f32)
            nc.scalar.activation(rt[:], sq[:], AF.Sqrt, bias=1.0, scale=-1.0)
            t1 = small.tile([128, rb], f32)
            nc.vector.tensor_scalar_mul(t1[:], xt_sb[:], SCALE * cos_m)
            t2 = small.tile([128, rb], f32)
            nc.vector.tensor_scalar_mul(t2[:], rt[:], SCALE * sin_m)
            lt = small.tile([128, rb], f32)
            nc.vector.tensor_sub(lt[:], t1[:], t2[:])
            e1 = small.tile([128, rb], f32)
            nc.scalar.activation(e1[:], lt[:], AF.Exp, bias=kbias[:], scale=1.0)
            e0 = small.tile([128, rb], f32)
            nc.scalar.activation(e0[:], xt_sb[:], AF.Exp, bias=kbias[:], scale=SCALE)
            # corr = e1 - e0, written into acc column ct of each row-block
            nc.vector.tensor_sub(acc[:, ct :: ct + 1], e1[:], e0[:])

            # ---- bulk pass: exp(SCALE*x - K) summed along free axis ----
            for r in range(rb):
                for t in range(ct):
                    cos_tile = stream_pool.tile([128, f], f32, tag="stream")
                    i = r * ct + t
                    eng = nc.gpsimd if (split_dma_engines and i % 2) else nc.sync
                    eng.dma_start(
                        out=cos_tile[:],
                        in_=cos_ext[r * 128 : (r + 1) * 128, t * f : (t + 1) * f],
                    )
                    j = r * (ct + 1) + t
                    nc.scalar.activation(
                        cos_tile[:],
                        cos_tile[:],
                        AF.Exp,
                        bias=kbias[:],
                        scale=SCALE,
                        accum_out=acc[:, j : j + 1],
                    )

            # ---- S_true[p, r] = sum over the ct+1 columns of row-block r ----
            st = small.tile([128, rb], f32)
            acc_view = acc[:, :].rearrange("p (r t) -> p r t", t=ct + 1)
            nc.vector.reduce_sum(st[:], acc_view, axis=mybir.AxisListType.X)
            lg = small.tile([128, rb], f32)
            nc.scalar.activation(lg[:], st[:], AF.Ln)
            # loss = (lg + K) - lt, with fused per-partition row sum
            lossv = small.tile([128, rb], f32)
            rowsum = small.tile([128, 1], f32)
            nc.vector.scalar_tensor_tensor(
                lossv[:],
                lg[:],
                K_SHIFT,
                lt[:],
                op0=mybir.AluOpType.add,
                op1=mybir.AluOpType.subtract,
                accum_out=rowsum[:],
            )
            # ---- partition-sum via TensorE; ones = 1/B so ps is the mean ----
            ps = psum.tile([1, 1], f32)
            nc.tensor.matmul(ps[:], ones[:], rowsum[:])
            part = small.tile([1, 1], f32)
            nc.vector.tensor_copy(part[:], ps[:])

            if final_collective and allgather_final:
                # ---- AllGather the 8 partial means, sum locally ----
                cc_in = dram.tile([1, 1], f32)
                ag_out = dram.tile([1, n_cores], f32)
                nc.sync.dma_start(out=cc_in[:], in_=part[:])
                nc.gpsimd.collective_compute(
                    "AllGather",
                    mybir.AluOpType.bypass,
                    replica_groups=[list(range(n_cores))],
                    ins=[cc_in.opt()],
                    outs=[ag_out.opt()],
                )
                ag_sb = small.tile([1, n_cores], f32)
                nc.sync.dma_start(out=ag_sb[:], in_=ag_out[:])
                total = small.tile([1, 1], f32)
                nc.vector.reduce_sum(total[:], ag_sb[:], axis=mybir.AxisListType.X)
                nc.sync.dma_start(out=out_ext[:], in_=total[:])
            elif final_collective:
                # ---- AllReduce(add) the 8 partial means ----
                cc_in = dram.tile([1, 1], f32)
                cc_out = dram.tile([1, 1], f32)
                nc.sync.dma_start(out=cc_in[:], in_=part[:])
                nc.gpsimd.collective_compute(
                    "AllReduce",
                    mybir.AluOpType.add,
                    replica_groups=[list(range(n_cores))],
                    ins=[cc_in.opt()],
                    outs=[cc_out.opt()],
                )
                nc.sync.dma_start(out=out_ext[:], in_=cc_out[:])
            else:
                # partials summed on host
                nc.sync.dma_start(out=out_ext[:], in_=part[:])

    nc.compile()
    return nc


def make_in_maps(cosine, label, b_per=B_PER, n_cores=N_CORES):
    """Host-side sharding: batch-split cosine (zero copy) + gather target
    cosines, laid out [128, rb] to match the device row layout."""
    cosine = np.ascontiguousarray(np.asarray(cosine, dtype=np.float32))
    label = np.asarray(label).astype(np.int64)
    b = cosine.shape[0]
    rb = b_per // 128
    xt = cosine[np.arange(b), label]  # [B] f32
    in_maps = []
    for i in range(n_cores):
        shard = cosine[i * b_per : (i + 1) * b_per]
        xtc = np.ascontiguousarray(xt[i * b_per : (i + 1) * b_per].reshape(rb, 128).T)
        in_maps.append({"cosine": shard, "xt": xtc})
    return in_maps


def kernel(cosine, label):
    from concourse.bass_utils import run_bass_kernel_spmd

    if "nc" not in _CACHE:
        _CACHE["nc"] = build_bass()
    nc = _CACHE["nc"]
    in_maps = make_in_maps(cosine, label)
    res = run_bass_kernel_spmd(nc, in_maps, core_ids=list(range(N_CORES)))
    out = np.asarray(res.results[0]["out"], dtype=np.float32).reshape(())
    return out



# revision 2
# speedup vs baseline: 1.0923x; 1.0923x over previous
"""ArcFace loss on 8 TRN2 NeuronCores (batch-parallel Bass/Tile kernel).

Math: for non-target classes cos(arccos(x)) == x, so logits are just
SCALE*x everywhere except the B target entries, which get
SCALE*(x*cos(m) - sqrt(1-x^2)*sin(m)).  Since cosine < 0.99 strictly,
SCALE*0.99 is an upper bound of every logit, so a constant shift
K = SCALE*0.99 replaces the per-row max (logsumexp is shift-invariant)
and the [B, C] pass is a single streamed exp-accumulate:

    S_all[b]  = sum_c exp(SCALE*x[b,c] - K)           (device, streamed)
    lt[b]     = SCALE*(xt*cos(m) - sqrt(1-xt^2)*sin(m))
    S_true[b] = S_all - exp(SCALE*xt - K) + exp(lt - K)
    loss      = mean_b [ log(S_true) + K - lt ]

Sharding: batch dimension B=2048 -> 256 rows per core (zero-copy host
shards).  Each core streams its [256, 100000] f32 shard (102.4 MB)
through SBUF; ScalarE does exp + free-axis accumulation in a single
ACTIVATE per tile (accum_out), so the pass is purely DMA-bound
(~350 GB/s/core HBM -> ~292 us floor).  The margin correction for the
core's rows is computed up front (overlapped with streaming), the
local rows reduce to a partial mean on-device, and the 8 per-core
partial means are summed on the host (a 4-byte AllReduce measured
~22 us of pure tail latency; host-side unshard removes it).  The last
column tile of each row block is split into a 3/4 + 1/4 taper so the
post-stream ACTIVATE tail shrinks from ~5.5 us to ~1.4 us.
"""

import math

import numpy as np

B = 2048
C = 100000
N_CORES = 8
B_PER = B // N_CORES  # 256 rows per core
RB = B_PER // 128  # 2 row-blocks of 128 partitions
CT = 16  # col-tiles per row-block (last one split in two)
F = C // CT  # free dim per tile

MARGIN = 0.1
SCALE = 64.0
K_SHIFT = SCALE * 0.99  # upper bound of all logits; constant lse shift

_CACHE = {}


def build_bass(
    b_per=B_PER,
    c=C,
    ct=CT,
    n_cores=N_CORES,
    bufs=6,
    split_dma_engines=False,
    taper=True,
    final_collective=False,
    warmup_collective=False,
):
    """Build + compile the SPMD Bass graph for one core (all cores identical)."""
    import concourse.bacc as bacc
    import concourse.bass as bass
    import concourse.tile as tile
    from concourse import mybir

    f32 = mybir.dt.float32
    AF = mybir.ActivationFunctionType
    rb = b_per // 128
    f = c // ct
    cos_m = float(np.float32(math.cos(MARGIN)))
    sin_m = float(np.float32(math.sin(MARGIN)))

    # column chunks within one row-block: ct-1 full tiles + tapered last
    # tile (3/4 then 1/4) so the final ACT after the last DMA is short.
    chunks = [(t * f, f) for t in range(ct - 1)]
    if taper and f >= 8:
        f0 = (3 * f) // 4
        chunks.append(((ct - 1) * f, f0))
        chunks.append(((ct - 1) * f + f0, f - f0))
    else:
        chunks.append(((ct - 1) * f, f))
    nt = len(chunks)  # tiles per row-block
    slots = nt + 1  # + margin-correction column

    nc = bacc.Bacc(
        "TRN2",
        target_bir_lowering=False,
        debug=False,
        num_devices=n_cores,
    )
    cos_ext = nc.dram_tensor("cosine", [b_per, c], f32, kind="ExternalInput")
    xt_ext = nc.dram_tensor("xt", [128, rb], f32, kind="ExternalInput")
    out_ext = nc.dram_tensor("out", [1, 1], f32, kind="ExternalOutput")

    with tile.TileContext(nc) as tc:
        with (
            tc.tile_pool(name="stream", bufs=bufs) as stream_pool,
            tc.tile_pool(name="small", bufs=1) as small,
            tc.tile_pool(name="psum", bufs=1, space="PSUM") as psum,
            tc.tile_pool(name="dram", bufs=1, space="DRAM") as dram,
        ):
            # per-(row-block, chunk) partial row sums from ACT accum_out;
            # one extra column per row-block holds the margin correction so
            # a single reduce yields S_true directly.
            acc = small.tile([128, rb * slots], f32)

            # constant bias AP for exp(x*scale - K)
            kbias = small.tile([128, 1], f32)
            nc.vector.memset(kbias[:], -K_SHIFT)
            # matmul ones vector carries the 1/B mean scaling
            ones = small.tile([128, 1], f32)
            nc.vector.memset(ones[:], 1.0 / float(n_cores * b_per))

            if warmup_collective and final_collective:
                warm_sb = small.tile([1, 1], f32)
                nc.vector.memset(warm_sb[:], 0.0)
                warm_in = dram.tile([1, 1], f32)
                warm_out = dram.tile([1, 1], f32)
                nc.gpsimd.dma_start(out=warm_in[:], in_=warm_sb[:])
                nc.gpsimd.collective_compute(
                    "AllReduce",
                    mybir.AluOpType.add,
                    replica_groups=[list(range(n_cores))],
                    ins=[warm_in.opt()],
                    outs=[warm_out.opt()],
                )

            # ---- epilogue head: margin terms (independent of the stream);
            # xt comes in on the gpsimd queue so the sync queue's first
            # doorbell is stream tile 0.  Work overlaps the first DMAs.
            xt_sb = small.tile([128, rb], f32)
            nc.gpsimd.dma_start(out=xt_sb[:], in_=xt_ext[:])
            sq = small.tile([128, rb], f32)
            nc.vector.tensor_mul(sq[:], xt_sb[:], xt_sb[:])
            rt = small.tile([128, rb], f32)
            nc.scalar.activation(rt[:], sq[:], AF.Sqrt, bias=1.0, scale=-1.0)
            t1 = small.tile([128, rb], f32)
            nc.vector.tensor_scalar_mul(t1[:], xt_sb[:], SCALE * cos_m)
            t2 = small.tile([128, rb], f32)
            nc.vector.tensor_scalar_mul(t2[:], rt[:], SCALE * sin_m)
            lt = small.tile([128, rb], f32)
            nc.vector.tensor_sub(lt[:], t1[:], t2[:])
            e1 = small.tile([128, rb], f32)
            nc.scalar.activation(e1[:], lt[:], AF.Exp, bias=kbias[:], scale=1.0)
            e0 = small.tile([128, rb], f32)
            nc.scalar.activation(e0[:], xt_sb[:], AF.Exp, bias=kbias[:], scale=SCALE)
            # corr = e1 - e0, written into acc column nt of each row-block
            nc.vector.tensor_sub(acc[:, nt::slots], e1[:], e0[:])

            # ---- bulk pass: exp(SCALE*x - K) summed along free axis ----
            for r in range(rb):
                for t, (c0, cl) in enumerate(chunks):
                    cos_tile = stream_pool.tile([128, f], f32, tag="stream")
                    i = r * nt + t
                    eng = nc.gpsimd if (split_dma_engines and i % 2) else nc.sync
                    eng.dma_start(
                        out=cos_tile[:, :cl],
                        in_=cos_ext[r * 128 : (r + 1) * 128, c0 : c0 + cl],
                    )
                    j = r * slots + t
                    nc.scalar.activation(
                        cos_tile[:, :cl],
                        cos_tile[:, :cl],
                        AF.Exp,
                        bias=kbias[:],
                        scale=SCALE,
                        accum_out=acc[:, j : j + 1],
                    )

            # ---- S_true[p, r] = sum over the slots columns of row-block r ----
            st = small.tile([128, rb], f32)
            acc_view = acc[:, :].rearrange("p (r t) -> p r t", t=slots)
            nc.vector.reduce_sum(st[:], acc_view, axis=mybir.AxisListType.X)
            lg = small.tile([128, rb], f32)
            nc.scalar.activation(lg[:], st[:], AF.Ln)
            # loss = (lg + K) - lt, with fused per-partition row sum
            lossv = small.tile([128, rb], f32)
            rowsum = small.tile([128, 1], f32)
            nc.vector.scalar_tensor_tensor(
                lossv[:],
                lg[:],
                K_SHIFT,
                lt[:],
                op0=mybir.AluOpType.add,
                op1=mybir.AluOpType.subtract,
                accum_out=rowsum[:],
            )
            # ---- partition-sum via TensorE; ones = 1/B so ps is the mean ----
            ps = psum.tile([1, 1], f32)
            nc.tensor.matmul(ps[:], ones[:], rowsum[:])
            part = small.tile([1, 1], f32)
            nc.vector.tensor_copy(part[:], ps[:])

            if final_collective:
                cc_in = dram.tile([1, 1], f32)
                cc_out = dram.tile([1, 1], f32)
                nc.sync.dma_start(out=cc_in[:], in_=part[:])
                nc.gpsimd.collective_compute(
                    "AllReduce",
                    mybir.AluOpType.add,
                    replica_groups=[list(range(n_cores))],
                    ins=[cc_in.opt()],
                    outs=[cc_out.opt()],
                )
                nc.sync.dma_start(out=out_ext[:], in_=cc_out[:])
            else:
                # per-core partial mean; the 8 partials are summed on host
                nc.sync.dma_start(out=out_ext[:], in_=part[:])

    nc.compile()
    return nc


def make_in_maps(cosine, label, b_per=B_PER, n_cores=N_CORES):
    """Host-side sharding: batch-split cosine (zero copy) + gather target
    cosines, laid out [128, rb] to match the device row layout."""
    cosine = np.ascontiguousarray(np.asarray(cosine, dtype=np.float32))
    label = np.asarray(label).astype(np.int64)
    b = cosine.shape[0]
    rb = b_per // 128
    xt = cosine[np.arange(b), label]  # [B] f32
    in_maps = []
    for i in range(n_cores):
        shard = cosine[i * b_per : (i + 1) * b_per]
        xtc = np.ascontiguousarray(xt[i * b_per : (i + 1) * b_per].reshape(rb, 128).T)
        in_maps.append({"cosine": shard, "xt": xtc})
    return in_maps


def kernel(cosine, label):
    from concourse.bass_utils import run_bass_kernel_spmd

    if "nc" not in _CACHE:
        _CACHE["nc"] = build_bass()
    nc = _CACHE["nc"]
    in_maps = make_in_maps(cosine, label)
    res = run_bass_kernel_spmd(nc, in_maps, core_ids=list(range(N_CORES)))
    # unshard: each core returns its partial mean over its 256 rows
    total = 0.0
    for i in range(N_CORES):
        total += float(np.asarray(res.results[i]["out"], dtype=np.float32).reshape(()))
    return np.float32(total)
